# revision 4
# baseline (speedup 1.0000x reference)
"""Trainium2 Bass kernel for nn_Conditioned_Mlp (moe_routing).

Computation (reference):
    h      = relu(q @ W1[e] + b1[e])          [N, E, H]
    q_pred = h @ W2[e] + b2[e]                [N, E, D]
    gate   = softmax(concat(q, k) @ Wg + bg)  [N, E]
    out    = sum_e gate[:, e] * q_pred[:, :, e]

Sharding: pure data-parallel over N across 8 cores (2048 tokens/core);
all weights replicated.  Per core the kernel processes tokens in tiles
of 512, keeps h in transposed layout [H, tok] so layer-2 consumes it as
the stationary operand directly, and fuses gate/softmax/combine on-chip.
Matmuls run in bf16 (fp32 PSUM accumulation).

Scheduling notes (from NTFF trace analysis):
  * Layer 2 is a single h-sweep with all 8 PSUM banks live (dh x t4),
    so w2t chunk j's last read happens at h=4j+3 and the next expert's
    W2 DMA WAR-releases progressively from the START of layer 2 instead
    of all at once at its end.
  * All loads ride the Sync HWDGE queue, emitted per expert as
    [w1 hg0, w1 hg1, W2 x8 chunks, w1 hg2..7]: the two W1 groups the PE
    needs at the expert boundary sit AHEAD of the 8 MB W2 stream, whose
    chunk WARs pace it through the previous layer 2.  (W2 on the
    Activation queue was tried and regressed: its WAR waits head-of-line
    blocked the RELU stream and stalled layer 1 on psum drains.)
  * W1 is loaded one contiguous 1 MB DMA per hg group (host pre-packs
    [E, HG, 128, DC, 512]) - 8 sync-queue issues per expert instead of 64.
  * The output store is issued from the Activation queue one expert into
    the NEXT tile, where its y-drain dependency is already resolved, so
    it never blocks RELUs.
  * The gate runs transposed: Wg is the stationary operand, one [4, 512]
    logit psum per tile (17 full-width matmuls instead of 68 N=4 ones
    with per-matmul 128-col weight loads); softmax sums/normalization
    happen after 32x32 DVE block transposes.  Gate PE time drops ~4x.

Host-side work: dtype conversion to bf16 and weight-layout reordering so
every DMA the device issues is fully contiguous.
"""

import sys

sys.path.insert(0, "/opt/trn_rl_repo")

from contextlib import ExitStack

import ml_dtypes
import numpy as np

import concourse.bass as bass
import concourse.mybir as mybir
import concourse.tile as tile
from concourse import bacc
from concourse.bass import ds, ts
from concourse.bass_utils import run_bass_kernel_spmd

BF16 = mybir.dt.bfloat16
F32 = mybir.dt.float32
AF = mybir.ActivationFunctionType
ALU = mybir.AluOpType

N, D, E, H = 16384, 1024, 4, 4096
NCORES = 8
NT = N // NCORES  # tokens per core (2048)
TT = 512          # tokens per tile
NTT = NT // TT    # token tiles per core (4)
NT128 = TT // 128 # 128-token chunks per tile (4)
DC = D // 128     # contraction chunks over D (8)
HC = H // 128     # h-chunks (32)
HG = H // 512     # W1 streaming groups over H (8)

_CACHE = {}


def _build(trace_sim=False):
    nc = bacc.Bacc("TRN2", target_bir_lowering=False)

    # qtr[t, p, j, tok] = q[t*TT + tok, j*128 + p]  (host pre-transposed)
    q = nc.dram_tensor("qtr", [NTT, 128, DC, TT], BF16, kind="ExternalInput")
    k = nc.dram_tensor("ktr", [NTT, 128, DC, TT], BF16, kind="ExternalInput")
    # w1r[e, hg, p, d, s] = W1[e, d*128+p, hg*512+s]
    w1 = nc.dram_tensor("w1r", [E, HG, 128, DC, 512], BF16, kind="ExternalInput")
    # w2r[e, p, c, d] = W2[e, c*128+p, d]
    w2 = nc.dram_tensor("w2r", [E, 128, HC, D], BF16, kind="ExternalInput")
    # b1r[p, e*HC+c] = b1[e, c*128+p]
    b1 = nc.dram_tensor("b1r", [128, E * HC], F32, kind="ExternalInput")
    b2 = nc.dram_tensor("b2", [1, E * D], BF16, kind="ExternalInput")
    # wgr[p, j, g] = Wg[j*128+p, g]   (j < DC: q part; j >= DC: k part)
    wg = nc.dram_tensor("wgr", [128, 2 * DC, 4], BF16, kind="ExternalInput")
    bg = nc.dram_tensor("bg", [1, 4], BF16, kind="ExternalInput")
    out = nc.dram_tensor("out", [NT, D], F32, kind="ExternalOutput")

    with ExitStack() as ctx:
        tc = ctx.enter_context(tile.TileContext(nc, trace_sim=trace_sim))
        const = ctx.enter_context(tc.tile_pool(name="const", bufs=1))
        qkp = ctx.enter_context(tc.tile_pool(name="qk", bufs=2))
        w1p = ctx.enter_context(tc.tile_pool(name="w1p", bufs=3))
        w2p = ctx.enter_context(tc.tile_pool(name="w2p", bufs=1))
        htp = ctx.enter_context(tc.tile_pool(name="htp", bufs=1))
        yp = ctx.enter_context(tc.tile_pool(name="yp", bufs=2))
        gp = ctx.enter_context(tc.tile_pool(name="gp", bufs=4))
        psp = ctx.enter_context(tc.tile_pool(name="psp", bufs=8, space="PSUM"))

        ones = const.tile([1, 128], BF16)
        nc.vector.memset(ones, 1.0)
        ones_row = const.tile([1, 512], BF16)
        nc.vector.memset(ones_row, 1.0)
        wg_sb = const.tile([128, 2 * DC, 4], BF16)
        nc.sync.dma_start(out=wg_sb, in_=wg[:, :, :])
        b1_sb = const.tile([128, E * HC], F32)
        nc.sync.dma_start(out=b1_sb, in_=b1[:, :])
        b2_sb = const.tile([1, E, D], BF16)
        nc.sync.dma_start(out=b2_sb, in_=b2[:, :].rearrange("p (e d) -> p e d", e=E))
        bg_sb = const.tile([1, 4], BF16)
        nc.sync.dma_start(out=bg_sb, in_=bg[:, :])

        # qT lands first; kT is only needed once the gate matmuls run
        # (after expert 0's layer 1), so its DMA is deferred into the
        # first expert's hg loop to keep the startup critical path to
        # qT + one W1 group.
        qT0 = qkp.tile([128, DC, TT], BF16, tag="qT")
        kT0 = qkp.tile([128, DC, TT], BF16, tag="kT")
        nc.sync.dma_start(out=qT0[:, :, :], in_=q[0, :, :, :])
        qk_next = (qT0, kT0)
        pending_out = None  # (y, tok0) of the previous tile, stored late

        for t in range(NTT):
            tok0 = t * TT
            # qT[p, j, tok] = q[tok0+tok, j*128+p]
            qT, kT = qk_next
            gates = None  # computed after expert 0's layer 1

            y = yp.tile([128, NT128, D], F32, tag="y")

            # ---- experts
            for e in range(E):
                # W1 groups hg0+hg1 are what the PE needs first at the
                # expert boundary - they lead the 8 MB W2 stream on the
                # sync queue.
                w1ts = [None] * HG

                def load_w1(hg, e=e):
                    w1t = w1p.tile([128, DC, 512], BF16, tag="w1", name=f"w1_{hg}")
                    nc.sync.dma_start(out=w1t[:, :, :], in_=w1[e, hg])
                    w1ts[hg] = w1t

                load_w1(0)
                load_w1(1)
                # W2: 8 chunk DMAs; chunk j's WAR on the previous
                # expert's layer-2 reads releases at h=4j+3 of its
                # h-sweep, pacing this 8 MB through that layer 2.
                w2t = w2p.tile([128, HC, D], BF16, tag="w2")
                for j in range(8):
                    nc.sync.dma_start(
                        out=w2t[:, ds(j * 4, 4), :], in_=w2[e, :, ds(j * 4, 4), :]
                    )
                ht = htp.tile([128, HC, TT], BF16, tag="ht")
                # layer 1: hT[p, c, tok] = relu(q @ W1 + b1)[tok, c*128+p]
                for hg in range(HG):
                    if t == 0 and e == 0 and hg == 3:
                        nc.sync.dma_start(out=kT0[:, :, :], in_=k[0, :, :, :])
                    if e == E - 1 and t + 1 < NTT:
                        # prefetch next token tile's q/k mid-layer-1 of
                        # the last expert, split so neither transfer
                        # delays this expert's W1 group stream.
                        if hg == 3:
                            qTn = qkp.tile([128, DC, TT], BF16, tag="qT")
                            nc.sync.dma_start(out=qTn[:, :, :], in_=q[t + 1, :, :, :])
                        if hg == 6:
                            kTn = qkp.tile([128, DC, TT], BF16, tag="kT")
                            nc.sync.dma_start(out=kTn[:, :, :], in_=k[t + 1, :, :, :])
                            qk_next = (qTn, kTn)
                    if hg + 2 < HG:
                        load_w1(hg + 2)
                    w1t = w1ts[hg]
                    for hs in range(4):
                        hc = hg * 4 + hs
                        p1 = psp.tile([128, TT], F32, tag="ps")
                        for d in range(DC):
                            nc.tensor.matmul(
                                p1,
                                lhsT=w1t[:, d, ds(hs * 128, 128)],
                                rhs=qT[:, d, :],
                                start=(d == 0),
                                stop=(d == DC - 1),
                            )
                        nc.scalar.activation(
                            ht[:, hc, :], p1, AF.Relu,
                            bias=b1_sb[:, e * HC + hc : e * HC + hc + 1],
                        )

                if e == 0:
                    # ---- gate: softmax(concat(q,k) @ Wg + bg).
                    # Transposed orientation: Wg slices are the
                    # stationary operand (4-column weight loads), tokens
                    # stream, one [4, 512] logit psum for the tile.
                    pl = psp.tile([4, 512], F32, tag="ps")
                    for j in range(DC):
                        nc.tensor.matmul(
                            pl, lhsT=wg_sb[:, j, :], rhs=qT[:, j, :],
                            start=(j == 0), stop=False,
                        )
                    for j in range(DC):
                        nc.tensor.matmul(
                            pl, lhsT=wg_sb[:, DC + j, :], rhs=kT[:, j, :],
                            start=False, stop=False,
                        )
                    # + bg broadcast over tokens: outer(bg, ones_row)
                    nc.tensor.matmul(
                        pl, lhsT=bg_sb, rhs=ones_row, start=False, stop=True
                    )
                    # logits are ~N(0,1); exp cannot overflow, skip
                    # max-subtraction
                    gexpT = gp.tile([32, TT], F32, tag="gexpT")
                    nc.scalar.activation(gexpT[0:4, :], pl, AF.Exp)
                    # 32x32 DVE block transposes -> [128tok, 4] per
                    # 128-token chunk, then sum/normalize on DVE.
                    gates = []
                    for t4 in range(NT128):
                        gexp4 = gp.tile([128, 32], F32, tag="gexp4")
                        for pb in range(4):
                            nc.vector.transpose(
                                out=gexp4[pb * 32 : (pb + 1) * 32, 0:32],
                                in_=gexpT[:, ds(t4 * 128 + pb * 32, 32)],
                            )
                        s2 = gp.tile([128, 2], F32, tag="s2")
                        nc.vector.scalar_tensor_tensor(
                            out=s2, in0=gexp4[:, 0:2], scalar=1.0,
                            in1=gexp4[:, 2:4], op0=ALU.mult, op1=ALU.add,
                        )
                        gsum = gp.tile([128, 1], F32, tag="gsum")
                        nc.vector.scalar_tensor_tensor(
                            out=gsum, in0=s2[:, 0:1], scalar=1.0,
                            in1=s2[:, 1:2], op0=ALU.mult, op1=ALU.add,
                        )
                        grec = gp.tile([128, 1], F32, tag="grec")
                        nc.vector.reciprocal(grec, gsum)
                        gate = gp.tile([128, 4], F32, tag="gate")
                        nc.vector.tensor_scalar_mul(gate, gexp4[:, 0:4], grec)
                        gates.append(gate)

                if e == 1 and pending_out is not None:
                    # previous tile's store: issued from the Activation
                    # queue here, where its y-drain dependency is long
                    # resolved, so it neither blocks RELUs nor delays
                    # boundary-critical loads on the sync queue.
                    py, ptok0 = pending_out
                    nc.scalar.dma_start(
                        out=out[ptok0 : ptok0 + TT, :].rearrange(
                            "(c p) d -> p c d", p=128
                        ),
                        in_=py[:, :, :],
                    )
                    pending_out = None

                # layer 2 + gated accumulation into y: one h-sweep with
                # all 8 PSUM banks live (dh x t4), so w2t chunk j's last
                # read is at h=4j+3 and the next expert's W2 DMA starts
                # flowing 1/8th of the way into this loop.
                p2s = []
                for _i in range(2 * NT128):
                    p2s.append(psp.tile([128, 512], F32, tag="ps", name=f"p2_{_i}"))
                for h in range(HC):
                    for t4 in range(NT128):
                        for dh in range(2):
                            nc.tensor.matmul(
                                p2s[dh * NT128 + t4],
                                lhsT=ht[:, h, ts(t4, 128)],
                                rhs=w2t[:, h, ds(dh * 512, 512)],
                                start=(h == 0),
                                stop=False,
                            )
                for dh in range(2):
                    for t4 in range(NT128):
                        i = dh * NT128 + t4
                        # + b2[e] broadcast over tokens (K=1 ones matmul)
                        nc.tensor.matmul(
                            p2s[i], lhsT=ones,
                            rhs=b2_sb[:1, e, ds(dh * 512, 512)],
                            start=False, stop=True,
                        )
                        g_col = gates[t4][:, e : e + 1]
                        ysl = y[:, t4, ds(dh * 512, 512)]
                        if e == 0:
                            nc.vector.tensor_scalar_mul(ysl, p2s[i], g_col)
                        else:
                            nc.vector.scalar_tensor_tensor(
                                out=ysl, in0=p2s[i], scalar=g_col, in1=ysl,
                                op0=ALU.mult, op1=ALU.add,
                            )

            pending_out = (y, tok0)

        py, ptok0 = pending_out
        nc.scalar.dma_start(
            out=out[ptok0 : ptok0 + TT, :].rearrange("(c p) d -> p c d", p=128),
            in_=py[:, :, :],
        )

    nc.compile()
    return nc


def _get_nc():
    if "nc" not in _CACHE:
        _CACHE["nc"] = _build()
    return _CACHE["nc"]


def _prep_inputs(q, k, W1, b1, W2, b2, Wg, bg):
    bf16 = ml_dtypes.bfloat16
    q = np.asarray(q, dtype=np.float32)
    k = np.asarray(k, dtype=np.float32)
    W1 = np.asarray(W1, dtype=np.float32)
    b1 = np.asarray(b1, dtype=np.float32)
    W2 = np.asarray(W2, dtype=np.float32)
    b2 = np.asarray(b2, dtype=np.float32)
    Wg = np.asarray(Wg, dtype=np.float32)
    bg = np.asarray(bg, dtype=np.float32)

    # per-core pre-transposed q/k: [NTT, 128, DC, TT]
    def tr(x):
        xc = x.astype(bf16).reshape(NCORES, NTT, TT, DC, 128)
        return np.ascontiguousarray(xc.transpose(0, 1, 4, 3, 2))

    qtr = tr(q)
    ktr = tr(k)
    w1r = np.ascontiguousarray(
        W1.astype(bf16).reshape(E, DC, 128, HG, 512).transpose(0, 3, 2, 1, 4)
    )
    w2r = np.ascontiguousarray(
        W2.astype(bf16).reshape(E, HC, 128, D).transpose(0, 2, 1, 3)
    )
    b1r = np.ascontiguousarray(
        b1.reshape(E, HC, 128).transpose(2, 0, 1).reshape(128, E * HC)
    )
    wgr = np.ascontiguousarray(
        Wg.astype(bf16).reshape(2 * DC, 128, 4).transpose(1, 0, 2)
    )
    bgr = np.ascontiguousarray(bg.astype(bf16).reshape(1, 4))

    in_maps = []
    for c in range(NCORES):
        in_maps.append(
            {
                "qtr": qtr[c],
                "ktr": ktr[c],
                "w1r": w1r,
                "w2r": w2r,
                "b1r": b1r,
                "b2": np.ascontiguousarray(b2.astype(bf16).reshape(1, E * D)),
                "wgr": wgr,
                "bg": bgr,
            }
        )
    return in_maps


def run(inputs, trace=False):
    """Run the kernel; returns (output, BassKernelResults)."""
    in_maps = _prep_inputs(**inputs)
    res = run_bass_kernel_spmd(
        _get_nc(), in_maps, core_ids=list(range(NCORES)), trace=trace
    )
    out = np.concatenate([r["out"] for r in res.results], axis=0)
    return out, res


def kernel(**inputs):
    out, _ = run(inputs, trace=False)
    return out


# revision 13
# speedup vs baseline: 1.1266x; 1.1266x over previous
"""Trainium2 Bass kernel for nn_Conditioned_Mlp (moe_routing).

Computation (reference):
    h      = relu(q @ W1[e] + b1[e])          [N, E, H]
    q_pred = h @ W2[e] + b2[e]                [N, E, D]
    gate   = softmax(concat(q, k) @ Wg + bg)  [N, E]
    out    = sum_e gate[:, e] * q_pred[:, :, e]

Sharding: pure data-parallel over N across 8 cores (2048 tokens/core);
all weights replicated.  Per core the kernel processes tokens in tiles
of 512, keeps h in transposed layout [H, tok] so layer-2 consumes it as
the stationary operand directly, and fuses gate/softmax/combine on-chip.
Matmuls run in bf16 (fp32 PSUM accumulation).

Scheduling notes (from NTFF trace analysis):
  * Layer 2 is a single h-sweep with all 8 PSUM banks live (dh x t4),
    so w2t chunk j's last read happens at h=4j+3 and the next expert's
    W2 DMA WAR-releases progressively from the START of layer 2 instead
    of all at once at its end.
  * All loads ride the Sync HWDGE queue, emitted per expert as
    [w1 hg0, w1 hg1, W2 x8 chunks, w1 hg2..7]: the two W1 groups the PE
    needs at the expert boundary sit AHEAD of the 8 MB W2 stream, whose
    chunk WARs pace it through the previous layer 2.  (W2 on the
    Activation queue was tried and regressed: its WAR waits head-of-line
    blocked the RELU stream and stalled layer 1 on psum drains.)
  * W1 is loaded one contiguous 1 MB DMA per hg group (host pre-packs
    [E, HG, 128, DC, 512]) - 8 sync-queue issues per expert instead of 64.
  * The output store is issued from the Activation queue one expert into
    the NEXT tile, where its y-drain dependency is already resolved, so
    it never blocks RELUs.
  * The gate runs transposed: Wg is the stationary operand, one [4, 512]
    logit psum per tile (17 full-width matmuls instead of 68 N=4 ones
    with per-matmul 128-col weight loads); softmax sums/normalization
    happen after 32x32 DVE block transposes.  Gate PE time drops ~4x.

Host-side work: dtype conversion to bf16 and weight-layout reordering so
every DMA the device issues is fully contiguous.
"""

import sys

sys.path.insert(0, "/opt/trn_rl_repo")

from contextlib import ExitStack

import ml_dtypes
import numpy as np

import concourse.bass as bass
import concourse.mybir as mybir
import concourse.tile as tile
from concourse import bacc
from concourse.bass import ds, ts
from concourse.bass_utils import run_bass_kernel_spmd

BF16 = mybir.dt.bfloat16
F32 = mybir.dt.float32
AF = mybir.ActivationFunctionType
ALU = mybir.AluOpType

N, D, E, H = 16384, 1024, 4, 4096
NCORES = 8
NT = N // NCORES  # tokens per core (2048)
TT = 512          # tokens per tile
NTT = NT // TT    # token tiles per core (4)
NT128 = TT // 128 # 128-token chunks per tile (4)
DC = D // 128     # contraction chunks over D (8)
HC = H // 128     # h-chunks (32)
HG = H // 512     # W1 streaming groups over H (8)

_CACHE = {}


def _build(trace_sim=False):
    nc = bacc.Bacc("TRN2", target_bir_lowering=False)

    # qtr[t, p, j, tok] = q[t*TT + tok, j*128 + p]  (host pre-transposed)
    q = nc.dram_tensor("qtr", [NTT, 128, DC, TT], BF16, kind="ExternalInput")
    k = nc.dram_tensor("ktr", [NTT, 128, DC, TT], BF16, kind="ExternalInput")
    # w1r[e, hg, p, d, s] = W1[e, d*128+p, hg*512+s]
    w1 = nc.dram_tensor("w1r", [E, HG, 128, DC, 512], BF16, kind="ExternalInput")
    # w2r[e, p, c, d] = W2[e, c*128+p, d]
    w2 = nc.dram_tensor("w2r", [E, 128, HC, D], BF16, kind="ExternalInput")
    # b1r[p, e*HC+c] = b1[e, c*128+p]
    b1 = nc.dram_tensor("b1r", [128, E * HC], F32, kind="ExternalInput")
    b2 = nc.dram_tensor("b2", [1, E * D], BF16, kind="ExternalInput")
    # wgr[p, j, g] = Wg[j*128+p, g]   (j < DC: q part; j >= DC: k part)
    wg = nc.dram_tensor("wgr", [128, 2 * DC, 4], BF16, kind="ExternalInput")
    bg = nc.dram_tensor("bg", [1, 4], BF16, kind="ExternalInput")
    out = nc.dram_tensor("out", [NT, D], F32, kind="ExternalOutput")

    with ExitStack() as ctx:
        tc = ctx.enter_context(tile.TileContext(nc, trace_sim=trace_sim))
        const = ctx.enter_context(tc.tile_pool(name="const", bufs=1))
        qkp = ctx.enter_context(tc.tile_pool(name="qk", bufs=2))
        # hg0+hg1 of each expert live in their own 2-slot pool and are
        # loaded a full expert ahead: their slot WAR then points two
        # experts back (long released), sidestepping Tile's conservative
        # slot-recycle WAR that otherwise delays the boundary-critical
        # W1 groups until the next expert's L1 has already started.
        w1h = ctx.enter_context(tc.tile_pool(name="w1h", bufs=2))
        w1p = ctx.enter_context(tc.tile_pool(name="w1p", bufs=2))
        w2p = ctx.enter_context(tc.tile_pool(name="w2p", bufs=1))
        htp = ctx.enter_context(tc.tile_pool(name="htp", bufs=1))
        yp = ctx.enter_context(tc.tile_pool(name="yp", bufs=2))
        gp = ctx.enter_context(tc.tile_pool(name="gp", bufs=4))
        gep = ctx.enter_context(tc.tile_pool(name="gep", bufs=1))
        psp = ctx.enter_context(tc.tile_pool(name="psp", bufs=8, space="PSUM"))

        ones = const.tile([1, 128], BF16)
        nc.vector.memset(ones, 1.0)
        ones_row = const.tile([1, 512], BF16)
        nc.vector.memset(ones_row, 1.0)
        wg_sb = const.tile([128, 2 * DC, 4], BF16)
        nc.sync.dma_start(out=wg_sb, in_=wg[:, :, :])
        b1_sb = const.tile([128, E * HC], F32)
        nc.sync.dma_start(out=b1_sb, in_=b1[:, :])
        b2_sb = const.tile([1, E, D], BF16)
        nc.sync.dma_start(out=b2_sb, in_=b2[:, :].rearrange("p (e d) -> p e d", e=E))
        bg_sb = const.tile([1, 4], BF16)
        nc.sync.dma_start(out=bg_sb, in_=bg[:, :])

        # qT lands first; kT is only needed once the gate matmuls run
        # (after expert 0's layer 1), so its DMA is deferred into the
        # first expert's hg loop to keep the startup critical path to
        # qT + one W1 group.
        qT0 = qkp.tile([128, DC, TT], BF16, tag="qT")
        kT0 = qkp.tile([128, DC, TT], BF16, tag="kT")
        nc.sync.dma_start(out=qT0[:, :, :], in_=q[0, :, :, :])
        qk_next = (qT0, kT0)
        pending_out = None  # (y, tok0) of the previous tile, stored late

        def load_head(e_next):
            pair = []
            for hg in (0, 1):
                w1t = w1h.tile(
                    [128, DC, 512], BF16, tag="w1h", name=f"w1h_{hg}"
                )
                nc.sync.dma_start(out=w1t[:, :, :], in_=w1[e_next, hg])
                pair.append(w1t)
            return pair

        head_next = load_head(0)

        for t in range(NTT):
            tok0 = t * TT
            # qT[p, j, tok] = q[tok0+tok, j*128+p]
            qT, kT = qk_next
            gates = None  # computed after expert 0's layer 1

            y = yp.tile([128, NT128, D], F32, tag="y")

            # ---- experts
            for e in range(E):
                w1ts = [None] * HG
                w1ts[0], w1ts[1] = head_next  # loaded one expert ago

                def load_w1(hg, e=e):
                    w1t = w1p.tile([128, DC, 512], BF16, tag="w1", name=f"w1_{hg}")
                    nc.sync.dma_start(out=w1t[:, :, :], in_=w1[e, hg])
                    w1ts[hg] = w1t

                def load_w2(e=e):
                    # W2: 8 chunk DMAs; chunk j's WAR on the previous
                    # expert's layer-2 reads paces this 8 MB through
                    # that layer 2's h-sweep.
                    w2t = w2p.tile([128, HC, D], BF16, tag="w2")
                    for j in range(8):
                        nc.sync.dma_start(
                            out=w2t[:, ds(j * 4, 4), :],
                            in_=w2[e, :, ds(j * 4, 4), :],
                        )
                    return w2t

                if not (t == 0 and e == 0):
                    w2t = load_w2()
                ht = htp.tile([128, HC, TT], BF16, tag="ht")
                # layer 1: hT[p, c, tok] = relu(q @ W1 + b1)[tok, c*128+p]
                for hg in range(HG):
                    if t == 0 and e == 0 and hg == 3:
                        nc.sync.dma_start(out=kT0[:, :, :], in_=k[0, :, :, :])
                    if e == E - 1 and t + 1 < NTT:
                        # prefetch next token tile's q/k mid-layer-1 of
                        # the last expert, split so neither transfer
                        # delays this expert's W1 group stream.
                        if hg == 3:
                            qTn = qkp.tile([128, DC, TT], BF16, tag="qT")
                            nc.sync.dma_start(out=qTn[:, :, :], in_=q[t + 1, :, :, :])
                        if hg == 6:
                            kTn = qkp.tile([128, DC, TT], BF16, tag="kT")
                            nc.sync.dma_start(out=kTn[:, :, :], in_=k[t + 1, :, :, :])
                            qk_next = (qTn, kTn)
                    if 2 <= hg + 2 < HG:
                        load_w1(hg + 2)
                    w1t = w1ts[hg]
                    for hs in range(4):
                        hc = hg * 4 + hs
                        p1 = psp.tile([128, TT], F32, tag="ps")
                        for d in range(DC):
                            nc.tensor.matmul(
                                p1,
                                lhsT=w1t[:, d, ds(hs * 128, 128)],
                                rhs=qT[:, d, :],
                                start=(d == 0),
                                stop=(d == DC - 1),
                            )
                        nc.scalar.activation(
                            ht[:, hc, :], p1, AF.Relu,
                            bias=b1_sb[:, e * HC + hc : e * HC + hc + 1],
                        )

                if t == 0 and e == 0:
                    # first expert: W2 emitted after the W1 group loads
                    # so the 8 MB stream doesn't starve layer 1's groups
                    # during the cold start.
                    w2t = load_w2()
                # next expert's hg0+hg1: emitted here (after this
                # expert's own W1 loads) so their transfers neither
                # block nor trail the current expert's group stream.
                if e + 1 < E:
                    head_next = load_head(e + 1)
                elif t + 1 < NTT:
                    head_next = load_head(0)

                if e == 0:
                    # ---- gate: softmax(concat(q,k) @ Wg + bg).
                    # Transposed orientation: Wg slices are the
                    # stationary operand (4-column weight loads), tokens
                    # stream, one [4, 512] logit psum for the tile.
                    pl = psp.tile([4, 512], F32, tag="ps")
                    for j in range(DC):
                        nc.tensor.matmul(
                            pl, lhsT=wg_sb[:, j, :], rhs=qT[:, j, :],
                            start=(j == 0), stop=False,
                        )
                    for j in range(DC):
                        nc.tensor.matmul(
                            pl, lhsT=wg_sb[:, DC + j, :], rhs=kT[:, j, :],
                            start=False, stop=False,
                        )
                    # + bg broadcast over tokens: outer(bg, ones_row)
                    nc.tensor.matmul(
                        pl, lhsT=bg_sb, rhs=ones_row, start=False, stop=True
                    )
                    # logits are ~N(0,1); exp cannot overflow, skip
                    # max-subtraction
                    gexpT = gep.tile([32, TT], F32, tag="gexpT")
                    nc.scalar.activation(gexpT[0:4, :], pl, AF.Exp)
                    # 32x32 DVE block transposes -> [128tok, 4] per
                    # 128-token chunk, then sum/normalize on DVE.
                    gates = []
                    for t4 in range(NT128):
                        gexp4 = gp.tile([128, 32], F32, tag="gexp4")
                        for pb in range(4):
                            nc.vector.transpose(
                                out=gexp4[pb * 32 : (pb + 1) * 32, 0:32],
                                in_=gexpT[:, ds(t4 * 128 + pb * 32, 32)],
                            )
                        s2 = gp.tile([128, 2], F32, tag="s2")
                        nc.vector.scalar_tensor_tensor(
                            out=s2, in0=gexp4[:, 0:2], scalar=1.0,
                            in1=gexp4[:, 2:4], op0=ALU.mult, op1=ALU.add,
                        )
                        gsum = gp.tile([128, 1], F32, tag="gsum")
                        nc.vector.scalar_tensor_tensor(
                            out=gsum, in0=s2[:, 0:1], scalar=1.0,
                            in1=s2[:, 1:2], op0=ALU.mult, op1=ALU.add,
                        )
                        grec = gp.tile([128, 1], F32, tag="grec")
                        nc.vector.reciprocal(grec, gsum)
                        gate = gp.tile([128, 4], F32, tag="gate")
                        nc.vector.tensor_scalar_mul(gate, gexp4[:, 0:4], grec)
                        gates.append(gate)

                if e == 1 and pending_out is not None:
                    # previous tile's store: issued from the Activation
                    # queue here, where its y-drain dependency is long
                    # resolved, so it neither blocks RELUs nor delays
                    # boundary-critical loads on the sync queue.
                    py, ptok0 = pending_out
                    nc.scalar.dma_start(
                        out=out[ptok0 : ptok0 + TT, :].rearrange(
                            "(c p) d -> p c d", p=128
                        ),
                        in_=py[:, :, :],
                    )
                    pending_out = None

                # layer 2 + gated accumulation into y: one h-sweep with
                # all 8 PSUM banks live (dh x t4), so w2t chunk j's last
                # read is at h=4j+3 and the next expert's W2 DMA starts
                # flowing 1/8th of the way into this loop.
                p2s = []
                for _i in range(2 * NT128):
                    p2s.append(psp.tile([128, 512], F32, tag="ps", name=f"p2_{_i}"))
                for h in range(HC):
                    for t4 in range(NT128):
                        for dh in range(2):
                            nc.tensor.matmul(
                                p2s[dh * NT128 + t4],
                                lhsT=ht[:, h, ts(t4, 128)],
                                rhs=w2t[:, h, ds(dh * 512, 512)],
                                start=(h == 0),
                                stop=False,
                            )
                for dh in range(2):
                    for t4 in range(NT128):
                        i = dh * NT128 + t4
                        # + b2[e] broadcast over tokens (K=1 ones matmul)
                        nc.tensor.matmul(
                            p2s[i], lhsT=ones,
                            rhs=b2_sb[:1, e, ds(dh * 512, 512)],
                            start=False, stop=True,
                        )
                        g_col = gates[t4][:, e : e + 1]
                        ysl = y[:, t4, ds(dh * 512, 512)]
                        if e == 0:
                            nc.vector.tensor_scalar_mul(ysl, p2s[i], g_col)
                        else:
                            nc.vector.scalar_tensor_tensor(
                                out=ysl, in0=p2s[i], scalar=g_col, in1=ysl,
                                op0=ALU.mult, op1=ALU.add,
                            )
                        if t == NTT - 1 and e == E - 1 and dh == 1:
                            # last tile: store each 128-token quarter as
                            # soon as its final drain lands, shortening
                            # the kernel tail.
                            nc.scalar.dma_start(
                                out=out[tok0 + t4 * 128 : tok0 + (t4 + 1) * 128, :],
                                in_=y[:, t4, :],
                            )

            if t < NTT - 1:
                pending_out = (y, tok0)

    nc.compile()
    return nc


def _get_nc():
    if "nc" not in _CACHE:
        _CACHE["nc"] = _build()
    return _CACHE["nc"]


def _prep_inputs(q, k, W1, b1, W2, b2, Wg, bg):
    bf16 = ml_dtypes.bfloat16
    q = np.asarray(q, dtype=np.float32)
    k = np.asarray(k, dtype=np.float32)
    W1 = np.asarray(W1, dtype=np.float32)
    b1 = np.asarray(b1, dtype=np.float32)
    W2 = np.asarray(W2, dtype=np.float32)
    b2 = np.asarray(b2, dtype=np.float32)
    Wg = np.asarray(Wg, dtype=np.float32)
    bg = np.asarray(bg, dtype=np.float32)

    # per-core pre-transposed q/k: [NTT, 128, DC, TT]
    def tr(x):
        xc = x.astype(bf16).reshape(NCORES, NTT, TT, DC, 128)
        return np.ascontiguousarray(xc.transpose(0, 1, 4, 3, 2))

    qtr = tr(q)
    ktr = tr(k)
    w1r = np.ascontiguousarray(
        W1.astype(bf16).reshape(E, DC, 128, HG, 512).transpose(0, 3, 2, 1, 4)
    )
    w2r = np.ascontiguousarray(
        W2.astype(bf16).reshape(E, HC, 128, D).transpose(0, 2, 1, 3)
    )
    b1r = np.ascontiguousarray(
        b1.reshape(E, HC, 128).transpose(2, 0, 1).reshape(128, E * HC)
    )
    wgr = np.ascontiguousarray(
        Wg.astype(bf16).reshape(2 * DC, 128, 4).transpose(1, 0, 2)
    )
    bgr = np.ascontiguousarray(bg.astype(bf16).reshape(1, 4))

    in_maps = []
    for c in range(NCORES):
        in_maps.append(
            {
                "qtr": qtr[c],
                "ktr": ktr[c],
                "w1r": w1r,
                "w2r": w2r,
                "b1r": b1r,
                "b2": np.ascontiguousarray(b2.astype(bf16).reshape(1, E * D)),
                "wgr": wgr,
                "bg": bgr,
            }
        )
    return in_maps


def run(inputs, trace=False):
    """Run the kernel; returns (output, BassKernelResults)."""
    in_maps = _prep_inputs(**inputs)
    res = run_bass_kernel_spmd(
        _get_nc(), in_maps, core_ids=list(range(NCORES)), trace=trace
    )
    out = np.concatenate([r["out"] for r in res.results], axis=0)
    return out, res


def kernel(**inputs):
    out, _ = run(inputs, trace=False)
    return out
